# revision 3
# baseline (speedup 1.0000x reference)
"""Trainium2 Bass kernel for a Neural CDE forward pass.

Model (see reference): 2000 fixed Euler steps of
    y_{t+1} = y_t + dt * einsum('bhd,bd->bh', tanh-MLP(y_t).reshape(B,H,D), dX_t)
with a 3-layer softplus MLP (32 -> 128 -> 128 -> 256/tanh), batch B=128,
followed by a linear readout.

Strategy:
  * Pure data parallel over 8 NeuronCores (16 batch elements per core).
  * Feature-major activation layout (features on partitions, batch on the
    free dim) so every layer is a single PE matmul with a constant lhsT.
  * The cubic-spline derivative dX (and the dt factor) is precomputed on
    the host for all 2000 steps, pre-broadcast to the 256-feature layout
    the einsum needs, and streamed to SBUF in big chunks.
  * softplus(x) = Ln(Exp(x) + 1): two ScalarE ops from the single
    natural_log_exp activation table (gen3 has no softplus entry).
  * tanh(v) = 2/(1+exp(-2v)) - 1: one ScalarE Exp + DVE reciprocal,
    with the affine part fused into the dX multiply (one DVE op).
  * y is never materialized per step.  PSUM bank `psum1` accumulates
    A @ y_t (A = F0) directly across all steps:  psum1 += [A A .. A] @ g_t
    where g_t = tanh(..) * (2 dX dt) in a d-major 256-feature layout.
    Sum_t g_t is accumulated in SBUF and folded into y_T once at the end.
"""

import os
import numpy as np

B = 128
NP_KNOTS = 128
D = 8
H = 32
WID = 128
NCLS = 10
T0, T1 = 0.0, 20.0
DT0 = 0.01
NUM_STEPS = 2000
NCORES = 8
BS = B // NCORES  # 16 batch per core

_F32 = np.float32


# --------------------------------------------------------------------------
# Host-side precompute
# --------------------------------------------------------------------------

def _spline_dx(ts, coeff_d, coeff_c, coeff_b, num_steps):
    """dX/dt at each Euler step start time, with the (clipped) dt folded in.

    Mirrors the reference computation in fp32.  Returns (S, B, D)."""
    t_grid = (ts[0] + _F32(DT0) * np.arange(num_steps, dtype=_F32)).astype(_F32)
    dts = np.minimum(_F32(DT0), ts[-1] - t_grid).astype(_F32)
    idx = np.clip(np.searchsorted(ts, t_grid, side="right") - 1, 0, NP_KNOTS - 2)
    fr = (t_grid - ts[idx]).astype(_F32)[None, :, None]
    dX = (coeff_b[:, idx] + _F32(2.0) * coeff_c[:, idx] * fr
          + _F32(3.0) * coeff_d[:, idx] * fr * fr)          # (B, S, D)
    dX = np.transpose(dX, (1, 0, 2)).astype(_F32)           # (S, B, D)
    return dX * dts[:, None, None]


def _dxb_layout(dx_core, steps_per_chunk):
    """(S, BS, D) -> (CH, 128, C*32) chunked, d-major, h-broadcast layout.

    Feature p = d*32 + h lives in col-block cb = d // 4 (d_local = d % 4...
    precisely: partition p in col-block cb holds global feature cb*128 + p,
    i.e. d = cb*4 + p//32, h = p % 32).  Includes the factor 2 used by the
    fused tanh DVE op."""
    S = dx_core.shape[0]
    C = steps_per_chunk
    CH = S // C
    # [s, j, d] -> [s, j, cb, dblk] with d = cb*4 + dblk
    tmp = dx_core.reshape(S, BS, 2, 4)
    # -> [s, dblk, cb, j]
    tmp = np.transpose(tmp, (0, 3, 2, 1))
    # broadcast over h (32): [s, dblk, h, cb, j]
    tmp = np.broadcast_to(tmp[:, :, None, :, :], (S, 4, 32, 2, BS))
    arr = tmp.reshape(S, 128, 32)                      # [s, p, cb*16 + j]
    arr = arr.reshape(CH, C, 128, 32).transpose(0, 2, 1, 3).reshape(CH, 128, C * 32)
    return np.ascontiguousarray(_F32(2.0) * arr)


def _host_weights(W0, b0, W1, b1, W2, b2, F0, f0, F1, f1, F2, f2, R, rb):
    """All constant tensors, already transposed/permuted for the kernel."""
    f32 = lambda a: np.ascontiguousarray(a, dtype=_F32)
    # d-major permutation of the 256 func-MLP output features
    p = np.arange(256)
    perm = (p % 32) * 8 + p // 32          # F2p[p] = F2[(p%32)*8 + p//32]
    F2p = F2[perm]
    f2p = f2[perm]
    W = {
        "ATt":   f32(np.tile(F0.T, (4, 1))),          # (128,128) lhsT for psum1 += [A..A] @ g
        "F1T":   f32(F1.T),                            # (128,128)
        "F2aT":  f32(F2p[:128].T),                     # (128,128)
        "F2bT":  f32(F2p[128:].T),                     # (128,128)
        "f2rows": f32(np.stack([f2p[:128], f2p[128:]])),   # (2,128) bias lhsT
        "Sel":   f32(np.tile(np.eye(32, dtype=_F32), (4, 1))),  # (128,32)
        "W0T":   f32(W0.T),                            # (8,128)
        "W1T":   f32(W1.T),                            # (128,128)
        "W2T":   f32(W2.T),                            # (128,32)
        "AW2T":  f32((F0 @ W2).T),                     # (128,128)
        "Ab2":   f32((F0 @ b2)[None, :]),              # (1,128)
        "RT":    f32(R.T),                             # (32,10)
        "b0c":   f32(b0[:, None]),                     # (128,1)
        "b1c":   f32(b1[:, None]),
        "f0c":   f32(f0[:, None]),
        "f1c":   f32(f1[:, None]),
        "b2c":   f32(b2[:, None]),                     # (32,1)
        "rbc":   f32(rb[:, None]),                     # (10,1)
        "ones2": f32(np.stack([np.r_[np.ones(16), np.zeros(16)],
                               np.r_[np.zeros(16), np.ones(16)]])),  # (2,32)
        "ones16": f32(np.ones((1, 16))),
    }
    return W


# --------------------------------------------------------------------------
# Bass kernel build
# --------------------------------------------------------------------------

_NC_CACHE = {}


def _build_nc(num_steps, steps_per_chunk):
    key = (num_steps, steps_per_chunk)
    if key in _NC_CACHE:
        return _NC_CACHE[key]

    import concourse.bacc as bacc
    import concourse.bass as bass
    import concourse.mybir as mybir
    import concourse.tile as tile
    from contextlib import ExitStack

    f32 = mybir.dt.float32
    AF = mybir.ActivationFunctionType
    OP = mybir.AluOpType

    S = num_steps
    C = steps_per_chunk
    assert S % C == 0
    CH = S // C

    nc = bacc.Bacc("TRN2", target_bir_lowering=False, debug=False)

    # ---- DRAM I/O ----
    dram = {}
    wshapes = {
        "ATt": (128, 128), "F1T": (128, 128), "F2aT": (128, 128),
        "F2bT": (128, 128), "f2rows": (2, 128), "Sel": (128, 32),
        "W0T": (8, 128), "W1T": (128, 128), "W2T": (128, 32),
        "AW2T": (128, 128), "Ab2": (1, 128), "RT": (32, 10),
        "b0c": (128, 1), "b1c": (128, 1), "f0c": (128, 1), "f1c": (128, 1),
        "b2c": (32, 1), "rbc": (10, 1), "ones2": (2, 32), "ones16": (1, 16),
    }
    for name, shp in wshapes.items():
        dram[name] = nc.dram_tensor(name, list(shp), f32, kind="ExternalInput")
    dram["x0"] = nc.dram_tensor("x0", [8, BS], f32, kind="ExternalInput")
    dram["dxb"] = nc.dram_tensor("dxb", [CH, 128, C * 32], f32, kind="ExternalInput")
    out_dram = nc.dram_tensor("logits", [NCLS, BS], f32, kind="ExternalOutput")

    with tile.TileContext(nc) as tc, ExitStack() as ctx:
        const = ctx.enter_context(tc.tile_pool(name="const", bufs=1))
        dxbp = ctx.enter_context(tc.tile_pool(name="dxbp", bufs=2))
        work = ctx.enter_context(tc.tile_pool(name="work", bufs=3))
        accp = ctx.enter_context(tc.tile_pool(name="accp", bufs=1))
        psum = ctx.enter_context(
            tc.tile_pool(name="psum", bufs=1, space="PSUM"))
        ptmp = ctx.enter_context(
            tc.tile_pool(name="ptmp", bufs=2, space="PSUM"))

        # ---- constants into SBUF ----
        ct = {}
        for name, shp in wshapes.items():
            ct[name] = const.tile(list(shp), f32, tag=name, name=f"c_{name}")
            nc.sync.dma_start(ct[name][:], dram[name][:])
        x0_t = const.tile([8, BS], f32, tag="x0")
        nc.sync.dma_start(x0_t[:], dram["x0"][:])

        # ---- persistent PSUM tiles ----
        psum1 = psum.tile([128, BS], f32, tag="psum1")   # A @ y_t accumulator
        psum2 = psum.tile([128, BS], f32, tag="psum2")
        psum3 = psum.tile([128, 2 * BS], f32, tag="psum3")
        psum_y = psum.tile([32, BS], f32, tag="psum_y")  # y_T (minus b2)

        g_acc = accp.tile([128, 2 * BS], f32, tag="g_acc")
        nc.vector.memset(g_acc[:], 0.0)

        def softplus(ps_in, bias_ap, out_tile):
            """out = ln(1 + exp(ps_in + bias)); two ACT ops, one table."""
            e = ptmp.tile([128, BS], f32, tag="ptmp")
            nc.scalar.activation(e[:], ps_in, AF.Exp, bias=bias_ap)
            nc.scalar.activation(out_tile[:], e[:], AF.Ln, bias=1.0)

        # ---- initial MLP: y0 = W2 @ sp(W1 @ sp(W0 @ x0 + b0) + b1) (+ b2) ----
        psA = ptmp.tile([128, BS], f32, tag="ptmp")
        nc.tensor.matmul(psA[:], ct["W0T"][:], x0_t[:], start=True, stop=True)
        hA = work.tile([128, BS], f32, tag="h1")
        softplus(psA[:], ct["b0c"][:], hA)
        psB = ptmp.tile([128, BS], f32, tag="ptmp")
        nc.tensor.matmul(psB[:], ct["W1T"][:], hA[:], start=True, stop=True)
        hB = work.tile([128, BS], f32, tag="h2")
        softplus(psB[:], ct["b1c"][:], hB)

        # psum_y <- W2 @ hB   (b2 is added at the end)
        nc.tensor.matmul(psum_y[:], ct["W2T"][:], hB[:], start=True, stop=False,
                         skip_group_check=True)
        # psum1 <- A @ y0 = (F0 @ W2) @ hB + F0 @ b2
        nc.tensor.matmul(psum1[:], ct["AW2T"][:], hB[:], start=True, stop=False,
                         skip_group_check=True)
        nc.tensor.matmul(psum1[:], ct["Ab2"][:], ct["ones16"][:],
                         start=False, stop=False, skip_group_check=True)

        # ---- the 2000-step Euler scan ----
        g_prev = None
        for ch in range(CH):
            dxb_t = dxbp.tile([128, C * 32], f32, tag="dxb")
            nc.sync.dma_start(dxb_t[:], dram["dxb"][ch])
            for c in range(C):
                t = ch * C + c
                if t > 0:
                    # psum1 += [A .. A] @ g_{t-1}   (both 128-col halves)
                    nc.tensor.matmul(psum1[:], ct["ATt"][:], g_prev[:, 0:BS],
                                     start=False, stop=False, skip_group_check=True)
                    nc.tensor.matmul(psum1[:], ct["ATt"][:], g_prev[:, BS:2 * BS],
                                     start=False, stop=False, skip_group_check=True)
                # layer 1: h1 = sp(psum1 + f0)
                h1 = work.tile([128, BS], f32, tag="h1")
                softplus(psum1[:], ct["f0c"][:], h1)
                # layer 2
                nc.tensor.matmul(psum2[:], ct["F1T"][:], h1[:], start=True, stop=True)
                h2 = work.tile([128, BS], f32, tag="h2")
                softplus(psum2[:], ct["f1c"][:], h2)
                # layer 3: psum3 = F2p @ h2 + f2p   (bias via K=2 matmul)
                nc.tensor.matmul(psum3[:], ct["f2rows"][:], ct["ones2"][:],
                                 start=True, stop=False, skip_group_check=True)
                nc.tensor.matmul(psum3[:, 0:BS], ct["F2aT"][:], h2[:],
                                 start=False, stop=False, skip_group_check=True)
                nc.tensor.matmul(psum3[:, BS:2 * BS], ct["F2bT"][:], h2[:],
                                 start=False, stop=True, skip_group_check=True)
                # tanh(z) * (2 dX dt)  =  (1/(1+exp(-2z)) - 0.5) * (4 dX dt) ... :
                #   t3 = exp(-2 z); w = min(1+t3, 1e30); r ~= 1/w;
                #   g  = (r - 0.5) * dxb2          (dxb2 = 2 dt dX, d-major)
                t3 = work.tile([128, 2 * BS], f32, tag="t3")
                nc.scalar.activation(t3[:], psum3[:], AF.Exp, scale=-2.0)
                w = work.tile([128, 2 * BS], f32, tag="w")
                nc.vector.tensor_scalar(w[:], t3[:], 1.0, 1.0e30, OP.add, OP.min)
                r = work.tile([128, 2 * BS], f32, tag="r")
                nc.vector.reciprocal_approx_fast(r[:], w[:])
                g = work.tile([128, 2 * BS], f32, tag="g")
                nc.vector.scalar_tensor_tensor(
                    g[:], r[:], -0.5, dxb_t[:, c * 32:(c + 1) * 32],
                    OP.add, OP.mult)
                nc.vector.scalar_tensor_tensor(
                    g_acc[:], g[:], 1.0, g_acc[:], OP.mult, OP.add)
                g_prev = g

        # ---- finish: y_T = y0 + Sel @ sum_t g_t ----
        nc.tensor.matmul(psum_y[:], ct["Sel"][:], g_acc[:, 0:BS],
                         start=False, stop=False, skip_group_check=True)
        nc.tensor.matmul(psum_y[:], ct["Sel"][:], g_acc[:, BS:2 * BS],
                         start=False, stop=True, skip_group_check=True)
        y_sb = work.tile([32, BS], f32, tag="y_sb")
        nc.scalar.activation(y_sb[:], psum_y[:], AF.Identity, bias=ct["b2c"][:])
        # readout
        psl = ptmp.tile([NCLS, BS], f32, tag="ptmp")
        nc.tensor.matmul(psl[:], ct["RT"][:], y_sb[:], start=True, stop=True)
        out_sb = work.tile([NCLS, BS], f32, tag="out_sb")
        nc.scalar.activation(out_sb[:], psl[:], AF.Identity, bias=ct["rbc"][:])
        nc.sync.dma_start(out_dram[:], out_sb[:])

    nc.compile()
    _NC_CACHE[key] = nc
    return nc


# --------------------------------------------------------------------------
# Public entry point
# --------------------------------------------------------------------------

def _prepare_inputs(ts, coeff_d, coeff_c, coeff_b, coeff_a,
                    W0, b0, W1, b1, W2, b2, F0, f0, F1, f1, F2, f2, R, rb,
                    num_steps, steps_per_chunk):
    ts = np.asarray(ts, dtype=_F32)
    coeff_a = np.asarray(coeff_a, dtype=_F32)
    dx = _spline_dx(ts, np.asarray(coeff_d, _F32), np.asarray(coeff_c, _F32),
                    np.asarray(coeff_b, _F32), num_steps)          # (S,B,D), dt folded
    W = _host_weights(*[np.asarray(a, _F32) for a in
                        (W0, b0, W1, b1, W2, b2, F0, f0, F1, f1, F2, f2, R, rb)])
    in_maps = []
    for core in range(NCORES):
        bs = slice(core * BS, (core + 1) * BS)
        m = dict(W)
        m["x0"] = np.ascontiguousarray(coeff_a[bs, 0, :].T)        # (8,16)
        m["dxb"] = _dxb_layout(dx[:, bs, :], steps_per_chunk)      # (CH,128,C*32)
        in_maps.append(m)
    return in_maps


def kernel(ts, coeff_d, coeff_c, coeff_b, coeff_a,
           W0, b0, W1, b1, W2, b2, F0, f0, F1, f1, F2, f2, R, rb):
    from concourse.bass_utils import run_bass_kernel_spmd

    num_steps = NUM_STEPS
    steps_per_chunk = 250
    nc = _build_nc(num_steps, steps_per_chunk)
    in_maps = _prepare_inputs(ts, coeff_d, coeff_c, coeff_b, coeff_a,
                              W0, b0, W1, b1, W2, b2, F0, f0, F1, f1, F2, f2,
                              R, rb, num_steps, steps_per_chunk)
    res = run_bass_kernel_spmd(nc, in_maps, list(range(NCORES)))
    logits = np.concatenate(
        [res.results[i]["logits"].T for i in range(NCORES)], axis=0)
    return np.ascontiguousarray(logits.astype(np.float32))


# revision 14
# speedup vs baseline: 2.6662x; 2.6662x over previous
"""Trainium2 Bass kernel for a Neural CDE forward pass.

Model (see reference): 2000 fixed Euler steps of
    y_{t+1} = y_t + dt * einsum('bhd,bd->bh', tanh-MLP(y_t).reshape(B,H,D), dX_t)
with a 3-layer softplus MLP (32 -> 128 -> 128 -> 256/tanh), batch B=128,
followed by a linear readout.

Strategy:
  * Pure data parallel over 8 NeuronCores (16 batch elements per core).
  * Feature-major activation layout (features on partitions, batch on the
    free dim) so every layer is a single PE matmul with a constant lhsT.
  * The cubic-spline derivative dX (and the dt factor) is precomputed on
    the host for all 2000 steps, pre-broadcast to the 256-feature layout
    the einsum needs, and streamed to SBUF in big chunks.
  * softplus(x) = Ln(Exp(x) + 1): two ScalarE ops from the single
    natural_log_exp activation table (gen3 has no softplus entry).
  * tanh(v) = 2/(1+exp(-2v)) - 1: one ScalarE Exp + DVE reciprocal,
    with the affine part fused into the dX multiply (one DVE op).
  * y is never materialized per step.  PSUM bank `psum1` accumulates
    A @ y_t (A = F0) directly across all steps:  psum1 += [A A .. A] @ g_t
    where g_t = tanh(..) * (2 dX dt) in a d-major 256-feature layout.
    Sum_t g_t is accumulated in SBUF and folded into y_T once at the end.
"""

import os
import numpy as np

B = 128
NP_KNOTS = 128
D = 8
H = 32
WID = 128
NCLS = 10
T0, T1 = 0.0, 20.0
DT0 = 0.01
NUM_STEPS = 2000
NCORES = 8
BS = B // NCORES  # 16 batch per core

_F32 = np.float32


# --------------------------------------------------------------------------
# Host-side precompute
# --------------------------------------------------------------------------

def _spline_dx(ts, coeff_d, coeff_c, coeff_b, num_steps):
    """dX/dt at each Euler step start time, with the (clipped) dt folded in.

    Mirrors the reference computation in fp32.  Returns (S, B, D)."""
    t_grid = (ts[0] + _F32(DT0) * np.arange(num_steps, dtype=_F32)).astype(_F32)
    dts = np.minimum(_F32(DT0), ts[-1] - t_grid).astype(_F32)
    idx = np.clip(np.searchsorted(ts, t_grid, side="right") - 1, 0, NP_KNOTS - 2)
    fr = (t_grid - ts[idx]).astype(_F32)[None, :, None]
    dX = (coeff_b[:, idx] + _F32(2.0) * coeff_c[:, idx] * fr
          + _F32(3.0) * coeff_d[:, idx] * fr * fr)          # (B, S, D)
    dX = np.transpose(dX, (1, 0, 2)).astype(_F32)           # (S, B, D)
    return dX * dts[:, None, None]


def _dxb_layout(dx_core, steps_per_chunk):
    """(S, BS, D) -> (CH, 128, C*32) chunked, d-major, h-broadcast layout.

    Feature p = d*32 + h lives in col-block cb = d // 4 (d_local = d % 4...
    precisely: partition p in col-block cb holds global feature cb*128 + p,
    i.e. d = cb*4 + p//32, h = p % 32).  Includes the factor 2 used by the
    fused tanh DVE op."""
    S = dx_core.shape[0]
    C = steps_per_chunk
    CH = S // C
    # [s, j, d] -> [s, j, cb, dblk] with d = cb*4 + dblk
    tmp = dx_core.reshape(S, BS, 2, 4)
    # -> [s, dblk, cb, j]
    tmp = np.transpose(tmp, (0, 3, 2, 1))
    # broadcast over h (32): [s, dblk, h, cb, j]
    tmp = np.broadcast_to(tmp[:, :, None, :, :], (S, 4, 32, 2, BS))
    arr = tmp.reshape(S, 128, 32)                      # [s, p, cb*16 + j]
    arr = arr.reshape(CH, C, 128, 32).transpose(0, 2, 1, 3).reshape(CH, 128, C * 32)
    return np.ascontiguousarray(_F32(2.0) * arr)


MM_DT = np.float16  # dtype of the per-step matmuls (fp16: 1 cyc/row + FWL)


def _host_weights(W0, b0, W1, b1, W2, b2, F0, f0, F1, f1, F2, f2, R, rb):
    """All constant tensors, already transposed/permuted for the kernel."""
    f32 = lambda a: np.ascontiguousarray(a, dtype=_F32)
    f16 = lambda a: np.ascontiguousarray(a, dtype=MM_DT)
    # d-major permutation of the 256 func-MLP output features
    p = np.arange(256)
    perm = (p % 32) * 8 + p // 32          # F2p[p] = F2[(p%32)*8 + p//32]
    F2p = F2[perm]
    f2p = f2[perm]
    W = {
        "ATt":   f16(np.tile(F0.T, (4, 1))),          # (128,128) lhsT for psum1 += [A..A] @ g
        "F1T":   f16(F1.T),                            # (128,128)
        "F2aT":  f16(F2p[:128].T),                     # (128,128)
        "F2bT":  f16(F2p[128:].T),                     # (128,128)
        "f2rows": f16(np.stack([f2p[:128], f2p[128:]])),   # (2,128) bias lhsT
        "Sel":   f16(np.tile(np.eye(32, dtype=_F32), (4, 1))),  # (128,32)
        "W0T":   f32(W0.T),                            # (8,128)
        "W1T":   f32(W1.T),                            # (128,128)
        "W2T":   f32(W2.T),                            # (128,32)
        "AW2T":  f32((F0 @ W2).T),                     # (128,128)
        "Ab2":   f32((F0 @ b2)[None, :]),              # (1,128)
        "RT":    f32(R.T),                             # (32,10)
        "b0c":   f32(b0[:, None]),                     # (128,1)
        "b1c":   f32(b1[:, None]),
        "f0c":   f32(f0[:, None]),
        "f1c":   f32(f1[:, None]),
        "b2c":   f32(b2[:, None]),                     # (32,1)
        "rbc":   f32(rb[:, None]),                     # (10,1)
        "ones2": f16(np.stack([np.r_[np.ones(16), np.zeros(16)],
                               np.r_[np.zeros(16), np.ones(16)]])),  # (2,32)
        "ones16": f32(np.ones((1, 16))),
    }
    return W


# --------------------------------------------------------------------------
# Bass kernel build
# --------------------------------------------------------------------------

_NC_CACHE = {}


def _build_nc(num_steps, steps_per_chunk):
    key = (num_steps, steps_per_chunk)
    if key in _NC_CACHE:
        return _NC_CACHE[key]

    import concourse.bacc as bacc
    import concourse.bass as bass
    import concourse.mybir as mybir
    import concourse.tile as tile
    from contextlib import ExitStack

    f32 = mybir.dt.float32
    mmdt = mybir.dt.from_np(np.dtype(MM_DT))
    AF = mybir.ActivationFunctionType
    OP = mybir.AluOpType

    # Pin the activation-function table: everything we use (Exp, Ln,
    # Identity) lives in natural_log_exp_and_others.  Without this the
    # table chooser may alternate tables between Exp and Ln, inserting a
    # ~1.3us ACT_TABLE_LOAD several times per step.  The act_func_set_id
    # is an index into the FULL ordered table list, so keep all names and
    # positions, but strip our functions from every other table so the
    # chooser has exactly one option.
    import concourse.hw_specs as hw_specs
    _full_tabs = hw_specs.get_activation_tables("gen3")
    _ours = {AF.Exp, AF.Ln, AF.Identity, AF.Copy}
    _pinned = {
        name: (set(funcs) if name == "natural_log_exp_and_others"
               else set(funcs) - _ours)
        for name, funcs in _full_tabs.items()
    }
    bacc.get_activation_tables = lambda arch: _pinned

    S = num_steps
    C = steps_per_chunk
    assert S % C == 0
    CH = S // C

    nc = bacc.Bacc("TRN2", target_bir_lowering=False, debug=False)

    # ---- DRAM I/O ----
    dram = {}
    wshapes = {
        "ATt": (128, 128), "F1T": (128, 128), "F2aT": (128, 128),
        "F2bT": (128, 128), "f2rows": (2, 128), "Sel": (128, 32),
        "W0T": (8, 128), "W1T": (128, 128), "W2T": (128, 32),
        "AW2T": (128, 128), "Ab2": (1, 128), "RT": (32, 10),
        "b0c": (128, 1), "b1c": (128, 1), "f0c": (128, 1), "f1c": (128, 1),
        "b2c": (32, 1), "rbc": (10, 1), "ones2": (2, 32), "ones16": (1, 16),
    }
    mm_names = {"ATt", "F1T", "F2aT", "F2bT", "f2rows", "Sel", "ones2"}
    for name, shp in wshapes.items():
        dt_ = mmdt if name in mm_names else f32
        dram[name] = nc.dram_tensor(name, list(shp), dt_, kind="ExternalInput")
    dram["x0"] = nc.dram_tensor("x0", [8, BS], f32, kind="ExternalInput")
    dram["dxb"] = nc.dram_tensor("dxb", [CH, 128, C * 32], f32, kind="ExternalInput")
    out_dram = nc.dram_tensor("logits", [NCLS, BS], f32, kind="ExternalOutput")

    with tile.TileContext(nc) as tc, ExitStack() as ctx:
        const = ctx.enter_context(tc.tile_pool(name="const", bufs=1))
        dxbp = ctx.enter_context(tc.tile_pool(name="dxbp", bufs=2))
        work = ctx.enter_context(tc.tile_pool(name="work", bufs=3))
        psum = ctx.enter_context(
            tc.tile_pool(name="psum", bufs=1, space="PSUM"))
        ptmp = ctx.enter_context(
            tc.tile_pool(name="ptmp", bufs=2, space="PSUM"))

        # ---- constants into SBUF ----
        ct = {}
        for name, shp in wshapes.items():
            dt_ = mmdt if name in mm_names else f32
            ct[name] = const.tile(list(shp), dt_, tag=name, name=f"c_{name}")
            nc.sync.dma_start(ct[name][:], dram[name][:])
        x0_t = const.tile([8, BS], f32, tag="x0")
        nc.sync.dma_start(x0_t[:], dram["x0"][:])

        # ---- persistent PSUM tiles ----
        psum1 = psum.tile([128, BS], f32, tag="psum1")   # A @ y_t accumulator
        psum2 = psum.tile([128, BS], f32, tag="psum2")
        psum3 = psum.tile([128, 2 * BS], f32, tag="psum3")
        psum_y = psum.tile([32, BS], f32, tag="psum_y")  # y_T (minus b2)

        def softplus(ps_in, bias_ap, out_tile):
            """out = ln(1 + exp(ps_in + bias)); two ACT ops, one table."""
            e = ptmp.tile([128, BS], f32, tag="ptmp")
            nc.scalar.activation(e[:], ps_in, AF.Exp, bias=bias_ap)
            nc.scalar.activation(out_tile[:], e[:], AF.Ln, bias=1.0)

        # ---- initial MLP: y0 = W2 @ sp(W1 @ sp(W0 @ x0 + b0) + b1) (+ b2) ----
        psA = ptmp.tile([128, BS], f32, tag="ptmp")
        nc.tensor.matmul(psA[:], ct["W0T"][:], x0_t[:], start=True, stop=True)
        hA = work.tile([128, BS], f32, tag="h1")
        softplus(psA[:], ct["b0c"][:], hA)
        psB = ptmp.tile([128, BS], f32, tag="ptmp")
        nc.tensor.matmul(psB[:], ct["W1T"][:], hA[:], start=True, stop=True)
        hB = work.tile([128, BS], f32, tag="h2")
        softplus(psB[:], ct["b1c"][:], hB)

        # psum_y <- W2 @ hB   (b2 is added at the end)
        nc.tensor.matmul(psum_y[:], ct["W2T"][:], hB[:], start=True, stop=False,
                         skip_group_check=True)
        # psum1 <- A @ y0 = (F0 @ W2) @ hB + F0 @ b2
        nc.tensor.matmul(psum1[:], ct["AW2T"][:], hB[:], start=True, stop=False,
                         skip_group_check=True)
        nc.tensor.matmul(psum1[:], ct["Ab2"][:], ct["ones16"][:],
                         start=False, stop=False, skip_group_check=True)

        # ---- the 2000-step Euler scan ----
        g_prev = None
        for ch in range(CH):
            dxb_t = dxbp.tile([128, C * 32], f32, tag="dxb")
            nc.sync.dma_start(dxb_t[:], dram["dxb"][ch])
            for c in range(C):
                t = ch * C + c
                if t > 0:
                    # psum1 += [A .. A] @ g_{t-1}   (both 128-col halves)
                    nc.tensor.matmul(psum1[:], ct["ATt"][:], g_prev[:, 0:BS],
                                     start=False, stop=False, skip_group_check=True)
                    nc.tensor.matmul(psum1[:], ct["ATt"][:], g_prev[:, BS:2 * BS],
                                     start=False, stop=False, skip_group_check=True)
                # layer 1: h1 = sp(psum1 + f0)
                h1 = work.tile([128, BS], mmdt, tag="h1s")
                softplus(psum1[:], ct["f0c"][:], h1)
                # layer 2
                nc.tensor.matmul(psum2[:], ct["F1T"][:], h1[:], start=True, stop=True)
                h2 = work.tile([128, BS], mmdt, tag="h2s")
                softplus(psum2[:], ct["f1c"][:], h2)
                # layer 3: psum3 = F2p @ h2 + f2p   (bias via K=2 matmul)
                nc.tensor.matmul(psum3[:], ct["f2rows"][:], ct["ones2"][:],
                                 start=True, stop=False, skip_group_check=True)
                nc.tensor.matmul(psum3[:, 0:BS], ct["F2aT"][:], h2[:],
                                 start=False, stop=False, skip_group_check=True)
                nc.tensor.matmul(psum3[:, BS:2 * BS], ct["F2bT"][:], h2[:],
                                 start=False, stop=True, skip_group_check=True)
                # tanh(z) * (2 dX dt)  =  (1/(1+exp(-2z)) - 0.5) * (4 dX dt) ... :
                #   t3 = exp(-2 z); w = min(1+t3, 1e30); r ~= 1/w;
                #   g  = (r - 0.5) * dxb2          (dxb2 = 2 dt dX, d-major)
                t3 = work.tile([128, 2 * BS], f32, tag="t3")
                nc.scalar.activation(t3[:], psum3[:], AF.Exp, scale=-2.0)
                w = work.tile([128, 2 * BS], f32, tag="w")
                nc.vector.tensor_scalar(w[:], t3[:], 1.0, 1.0e30, OP.add, OP.min)
                r = work.tile([128, 2 * BS], f32, tag="r")
                nc.vector.reciprocal_approx_fast(r[:], w[:])
                g = work.tile([128, 2 * BS], mmdt, tag="g")
                nc.vector.scalar_tensor_tensor(
                    g[:], r[:], -0.5, dxb_t[:, c * 32:(c + 1) * 32],
                    OP.add, OP.mult)
                # psum_y += Sel @ g  (y_T accumulates off the critical path)
                nc.tensor.matmul(psum_y[:], ct["Sel"][:], g[:, 0:BS],
                                 start=False, stop=False, skip_group_check=True)
                nc.tensor.matmul(psum_y[:], ct["Sel"][:], g[:, BS:2 * BS],
                                 start=False, stop=(t == S - 1),
                                 skip_group_check=True)
                g_prev = g

        # ---- finish: y_T = y0 + sum_t Sel @ g_t (already in psum_y) ----
        y_sb = work.tile([32, BS], f32, tag="y_sb")
        nc.scalar.activation(y_sb[:], psum_y[:], AF.Identity, bias=ct["b2c"][:])
        # readout
        psl = ptmp.tile([NCLS, BS], f32, tag="ptmp")
        nc.tensor.matmul(psl[:], ct["RT"][:], y_sb[:], start=True, stop=True)
        out_sb = work.tile([NCLS, BS], f32, tag="out_sb")
        nc.scalar.activation(out_sb[:], psl[:], AF.Identity, bias=ct["rbc"][:])
        nc.sync.dma_start(out_dram[:], out_sb[:])

    nc.compile()
    _NC_CACHE[key] = nc
    return nc


# --------------------------------------------------------------------------
# Public entry point
# --------------------------------------------------------------------------

def _prepare_inputs(ts, coeff_d, coeff_c, coeff_b, coeff_a,
                    W0, b0, W1, b1, W2, b2, F0, f0, F1, f1, F2, f2, R, rb,
                    num_steps, steps_per_chunk):
    ts = np.asarray(ts, dtype=_F32)
    coeff_a = np.asarray(coeff_a, dtype=_F32)
    dx = _spline_dx(ts, np.asarray(coeff_d, _F32), np.asarray(coeff_c, _F32),
                    np.asarray(coeff_b, _F32), num_steps)          # (S,B,D), dt folded
    W = _host_weights(*[np.asarray(a, _F32) for a in
                        (W0, b0, W1, b1, W2, b2, F0, f0, F1, f1, F2, f2, R, rb)])
    in_maps = []
    for core in range(NCORES):
        bs = slice(core * BS, (core + 1) * BS)
        m = dict(W)
        m["x0"] = np.ascontiguousarray(coeff_a[bs, 0, :].T)        # (8,16)
        m["dxb"] = _dxb_layout(dx[:, bs, :], steps_per_chunk)      # (CH,128,C*32)
        in_maps.append(m)
    return in_maps


def kernel(ts, coeff_d, coeff_c, coeff_b, coeff_a,
           W0, b0, W1, b1, W2, b2, F0, f0, F1, f1, F2, f2, R, rb):
    from concourse.bass_utils import run_bass_kernel_spmd

    num_steps = NUM_STEPS
    steps_per_chunk = 250
    nc = _build_nc(num_steps, steps_per_chunk)
    in_maps = _prepare_inputs(ts, coeff_d, coeff_c, coeff_b, coeff_a,
                              W0, b0, W1, b1, W2, b2, F0, f0, F1, f1, F2, f2,
                              R, rb, num_steps, steps_per_chunk)
    res = run_bass_kernel_spmd(nc, in_maps, list(range(NCORES)))
    logits = np.concatenate(
        [res.results[i]["logits"].T for i in range(NCORES)], axis=0)
    return np.ascontiguousarray(logits.astype(np.float32))


# revision 16
# speedup vs baseline: 2.8052x; 1.0521x over previous
"""Trainium2 Bass kernel for a Neural CDE forward pass.

Model (see reference): 2000 fixed Euler steps of
    y_{t+1} = y_t + dt * einsum('bhd,bd->bh', tanh-MLP(y_t).reshape(B,H,D), dX_t)
with a 3-layer softplus MLP (32 -> 128 -> 128 -> 256/tanh), batch B=128,
followed by a linear readout.

Strategy:
  * Pure data parallel over 8 NeuronCores (16 batch elements per core).
  * Feature-major activation layout (features on partitions, batch on the
    free dim) so every layer is a single PE matmul with a constant lhsT.
  * The cubic-spline derivative dX (and the dt factor) is precomputed on
    the host for all 2000 steps, pre-broadcast to the 256-feature layout
    the einsum needs, and streamed to SBUF in big chunks.
  * softplus(x) = Ln(Exp(x) + 1): two ScalarE ops from the single
    natural_log_exp activation table (gen3 has no softplus entry).
  * tanh(v) = 2/(1+exp(-2v)) - 1: one ScalarE Exp + DVE reciprocal,
    with the affine part fused into the dX multiply (one DVE op).
  * y is never materialized per step.  PSUM bank `psum1` accumulates
    A @ y_t (A = F0) directly across all steps:  psum1 += [A A .. A] @ g_t
    where g_t = tanh(..) * (2 dX dt) in a d-major 256-feature layout.
    Sum_t g_t is accumulated in SBUF and folded into y_T once at the end.
"""

import os
import numpy as np

B = 128
NP_KNOTS = 128
D = 8
H = 32
WID = 128
NCLS = 10
T0, T1 = 0.0, 20.0
DT0 = 0.01
NUM_STEPS = 2000
NCORES = 8
BS = B // NCORES  # 16 batch per core

_F32 = np.float32


# --------------------------------------------------------------------------
# Host-side precompute
# --------------------------------------------------------------------------

def _spline_dx(ts, coeff_d, coeff_c, coeff_b, num_steps):
    """dX/dt at each Euler step start time, with the (clipped) dt folded in.

    Mirrors the reference computation in fp32.  Returns (S, B, D)."""
    t_grid = (ts[0] + _F32(DT0) * np.arange(num_steps, dtype=_F32)).astype(_F32)
    dts = np.minimum(_F32(DT0), ts[-1] - t_grid).astype(_F32)
    idx = np.clip(np.searchsorted(ts, t_grid, side="right") - 1, 0, NP_KNOTS - 2)
    fr = (t_grid - ts[idx]).astype(_F32)[None, :, None]
    dX = (coeff_b[:, idx] + _F32(2.0) * coeff_c[:, idx] * fr
          + _F32(3.0) * coeff_d[:, idx] * fr * fr)          # (B, S, D)
    dX = np.transpose(dX, (1, 0, 2)).astype(_F32)           # (S, B, D)
    return dX * dts[:, None, None]


def _dxb_layout(dx_core, steps_per_chunk):
    """(S, BS, D) -> (CH, 128, C*32) chunked, d-major, h-broadcast layout.

    Feature p = d*32 + h lives in col-block cb = d // 4 (d_local = d % 4...
    precisely: partition p in col-block cb holds global feature cb*128 + p,
    i.e. d = cb*4 + p//32, h = p % 32).  Includes the factor 2 used by the
    fused tanh DVE op."""
    S = dx_core.shape[0]
    C = steps_per_chunk
    CH = S // C
    # [s, j, d] -> [s, j, cb, dblk] with d = cb*4 + dblk
    tmp = dx_core.reshape(S, BS, 2, 4)
    # -> [s, dblk, cb, j]
    tmp = np.transpose(tmp, (0, 3, 2, 1))
    # broadcast over h (32): [s, dblk, h, cb, j]
    tmp = np.broadcast_to(tmp[:, :, None, :, :], (S, 4, 32, 2, BS))
    arr = tmp.reshape(S, 128, 32)                      # [s, p, cb*16 + j]
    arr = arr.reshape(CH, C, 128, 32).transpose(0, 2, 1, 3).reshape(CH, 128, C * 32)
    return np.ascontiguousarray(_F32(2.0) * arr)


MM_DT = np.float16  # dtype of the per-step matmuls (fp16: 1 cyc/row + FWL)


def _host_weights(W0, b0, W1, b1, W2, b2, F0, f0, F1, f1, F2, f2, R, rb):
    """All constant tensors, already transposed/permuted for the kernel."""
    f32 = lambda a: np.ascontiguousarray(a, dtype=_F32)
    f16 = lambda a: np.ascontiguousarray(a, dtype=MM_DT)
    # d-major permutation of the 256 func-MLP output features
    p = np.arange(256)
    perm = (p % 32) * 8 + p // 32          # F2p[p] = F2[(p%32)*8 + p//32]
    F2p = F2[perm]
    f2p = f2[perm]
    W = {
        "ATt":   f16(np.tile(F0.T, (4, 1))),          # (128,128) lhsT for psum1 += [A..A] @ g
        "F1T":   f16(F1.T),                            # (128,128)
        "F2aT":  f16(F2p[:128].T),                     # (128,128)
        "F2bT":  f16(F2p[128:].T),                     # (128,128)
        "f2rows": f16(np.stack([f2p[:128], f2p[128:]])),   # (2,128) bias lhsT
        "Sel":   f16(np.tile(np.eye(32, dtype=_F32), (4, 1))),  # (128,32)
        "W0T":   f32(W0.T),                            # (8,128)
        "W1T":   f32(W1.T),                            # (128,128)
        "W2T":   f32(W2.T),                            # (128,32)
        "AW2T":  f32((F0 @ W2).T),                     # (128,128)
        "Ab2":   f32((F0 @ b2)[None, :]),              # (1,128)
        "RT":    f32(R.T),                             # (32,10)
        "b0c":   f32(b0[:, None]),                     # (128,1)
        "b1c":   f32(b1[:, None]),
        "f0c":   f32(f0[:, None]),
        "f1c":   f32(f1[:, None]),
        "b2c":   f32(b2[:, None]),                     # (32,1)
        "rbc":   f32(rb[:, None]),                     # (10,1)
        "ones2": f16(np.stack([np.r_[np.ones(16), np.zeros(16)],
                               np.r_[np.zeros(16), np.ones(16)]])),  # (2,32)
        "ones16": f32(np.ones((1, 16))),
    }
    return W


# --------------------------------------------------------------------------
# Bass kernel build
# --------------------------------------------------------------------------

_NC_CACHE = {}


def _build_nc(num_steps, steps_per_chunk):
    key = (num_steps, steps_per_chunk)
    if key in _NC_CACHE:
        return _NC_CACHE[key]

    import concourse.bacc as bacc
    import concourse.bass as bass
    import concourse.mybir as mybir
    import concourse.tile as tile
    from contextlib import ExitStack

    f32 = mybir.dt.float32
    mmdt = mybir.dt.from_np(np.dtype(MM_DT))
    AF = mybir.ActivationFunctionType
    OP = mybir.AluOpType

    # Pin the activation-function table: everything we use (Exp, Ln,
    # Identity) lives in natural_log_exp_and_others.  Without this the
    # table chooser may alternate tables between Exp and Ln, inserting a
    # ~1.3us ACT_TABLE_LOAD several times per step.  The act_func_set_id
    # is an index into the FULL ordered table list, so keep all names and
    # positions, but strip our functions from every other table so the
    # chooser has exactly one option.
    import concourse.hw_specs as hw_specs
    _full_tabs = hw_specs.get_activation_tables("gen3")
    _ours = {AF.Exp, AF.Ln, AF.Identity, AF.Copy}
    _pinned = {
        name: (set(funcs) if name == "natural_log_exp_and_others"
               else set(funcs) - _ours)
        for name, funcs in _full_tabs.items()
    }
    bacc.get_activation_tables = lambda arch: _pinned

    S = num_steps
    C = steps_per_chunk
    assert S % C == 0
    CH = S // C

    nc = bacc.Bacc("TRN2", target_bir_lowering=False, debug=False)

    # ---- DRAM I/O ----
    dram = {}
    wshapes = {
        "ATt": (128, 128), "F1T": (128, 128), "F2aT": (128, 128),
        "F2bT": (128, 128), "f2rows": (2, 128), "Sel": (128, 32),
        "W0T": (8, 128), "W1T": (128, 128), "W2T": (128, 32),
        "AW2T": (128, 128), "Ab2": (1, 128), "RT": (32, 10),
        "b0c": (128, 1), "b1c": (128, 1), "f0c": (128, 1), "f1c": (128, 1),
        "b2c": (32, 1), "rbc": (10, 1), "ones2": (2, 32), "ones16": (1, 16),
    }
    mm_names = {"ATt", "F1T", "F2aT", "F2bT", "f2rows", "Sel", "ones2"}
    for name, shp in wshapes.items():
        dt_ = mmdt if name in mm_names else f32
        dram[name] = nc.dram_tensor(name, list(shp), dt_, kind="ExternalInput")
    dram["x0"] = nc.dram_tensor("x0", [8, BS], f32, kind="ExternalInput")
    dram["dxb"] = nc.dram_tensor("dxb", [CH, 128, C * 32], f32, kind="ExternalInput")
    out_dram = nc.dram_tensor("logits", [NCLS, BS], f32, kind="ExternalOutput")

    with tile.TileContext(nc) as tc, ExitStack() as ctx:
        const = ctx.enter_context(tc.tile_pool(name="const", bufs=1))
        dxbp = ctx.enter_context(tc.tile_pool(name="dxbp", bufs=2))
        work = ctx.enter_context(tc.tile_pool(name="work", bufs=3))
        psum = ctx.enter_context(
            tc.tile_pool(name="psum", bufs=1, space="PSUM"))
        ptmp = ctx.enter_context(
            tc.tile_pool(name="ptmp", bufs=2, space="PSUM"))

        # ---- constants into SBUF ----
        ct = {}
        for name, shp in wshapes.items():
            dt_ = mmdt if name in mm_names else f32
            ct[name] = const.tile(list(shp), dt_, tag=name, name=f"c_{name}")
            nc.sync.dma_start(ct[name][:], dram[name][:])
        x0_t = const.tile([8, BS], f32, tag="x0")
        nc.sync.dma_start(x0_t[:], dram["x0"][:])

        # ---- persistent PSUM tiles ----
        psum1 = psum.tile([128, BS], f32, tag="psum1")   # A @ y_t accumulator
        psum2 = psum.tile([128, BS], f32, tag="psum2")
        psum3 = psum.tile([128, 2 * BS], f32, tag="psum3")
        psum_y = psum.tile([32, BS], f32, tag="psum_y")  # y_T (minus b2)

        def softplus(ps_in, bias_ap, out_tile):
            """out = ln(1 + exp(ps_in + bias)); two ACT ops, one table."""
            e = ptmp.tile([128, BS], f32, tag="ptmp")
            nc.scalar.activation(e[:], ps_in, AF.Exp, bias=bias_ap)
            nc.scalar.activation(out_tile[:], e[:], AF.Ln, bias=1.0)

        # ---- initial MLP: y0 = W2 @ sp(W1 @ sp(W0 @ x0 + b0) + b1) (+ b2) ----
        psA = ptmp.tile([128, BS], f32, tag="ptmp")
        nc.tensor.matmul(psA[:], ct["W0T"][:], x0_t[:], start=True, stop=True)
        hA = work.tile([128, BS], f32, tag="h1")
        softplus(psA[:], ct["b0c"][:], hA)
        psB = ptmp.tile([128, BS], f32, tag="ptmp")
        nc.tensor.matmul(psB[:], ct["W1T"][:], hA[:], start=True, stop=True)
        hB = work.tile([128, BS], f32, tag="h2")
        softplus(psB[:], ct["b1c"][:], hB)

        # psum_y <- W2 @ hB   (b2 is added at the end)
        nc.tensor.matmul(psum_y[:], ct["W2T"][:], hB[:], start=True, stop=False,
                         skip_group_check=True)
        # psum1 <- A @ y0 = (F0 @ W2) @ hB + F0 @ b2
        nc.tensor.matmul(psum1[:], ct["AW2T"][:], hB[:], start=True, stop=False,
                         skip_group_check=True)
        nc.tensor.matmul(psum1[:], ct["Ab2"][:], ct["ones16"][:],
                         start=False, stop=False, skip_group_check=True)

        # ---- the 2000-step Euler scan ----
        g_prev = None
        for ch in range(CH):
            dxb_t = dxbp.tile([128, C * 32], f32, tag="dxb")
            nc.sync.dma_start(dxb_t[:], dram["dxb"][ch])
            for c in range(C):
                t = ch * C + c
                if t > 0:
                    # psum1 += [A .. A] @ g_{t-1}   (both 128-col halves)
                    nc.tensor.matmul(psum1[:], ct["ATt"][:], g_prev[:, 0:BS],
                                     start=False, stop=False, skip_group_check=True)
                    nc.tensor.matmul(psum1[:], ct["ATt"][:], g_prev[:, BS:2 * BS],
                                     start=False, stop=False, skip_group_check=True)
                # layer 1: h1 = sp(psum1 + f0)
                h1 = work.tile([128, BS], mmdt, tag="h1s")
                softplus(psum1[:], ct["f0c"][:], h1)
                # layer 2
                nc.tensor.matmul(psum2[:], ct["F1T"][:], h1[:], start=True, stop=True)
                if t > 0:
                    # psum_y += Sel @ g_{t-1}; queued after mm2 so it runs in
                    # the PE bubble while ACT does layer-2 softplus, keeping
                    # it off the critical chain.
                    nc.tensor.matmul(psum_y[:], ct["Sel"][:], g_prev[:, 0:BS],
                                     start=False, stop=False, skip_group_check=True)
                    nc.tensor.matmul(psum_y[:], ct["Sel"][:], g_prev[:, BS:2 * BS],
                                     start=False, stop=False, skip_group_check=True)
                h2 = work.tile([128, BS], mmdt, tag="h2s")
                softplus(psum2[:], ct["f1c"][:], h2)
                # layer 3: psum3 = F2p @ h2 + f2p   (bias via K=2 matmul)
                nc.tensor.matmul(psum3[:], ct["f2rows"][:], ct["ones2"][:],
                                 start=True, stop=False, skip_group_check=True)
                nc.tensor.matmul(psum3[:, 0:BS], ct["F2aT"][:], h2[:],
                                 start=False, stop=False, skip_group_check=True)
                nc.tensor.matmul(psum3[:, BS:2 * BS], ct["F2bT"][:], h2[:],
                                 start=False, stop=True, skip_group_check=True)
                # tanh(z) * (2 dX dt)  =  (1/(1+exp(-2z)) - 0.5) * (4 dX dt) ... :
                #   t3 = exp(-2 z); w = min(1+t3, 1e30); r ~= 1/w;
                #   g  = (r - 0.5) * dxb2          (dxb2 = 2 dt dX, d-major)
                t3 = work.tile([128, 2 * BS], f32, tag="t3")
                nc.scalar.activation(t3[:], psum3[:], AF.Exp, scale=-2.0)
                w = work.tile([128, 2 * BS], f32, tag="w")
                nc.vector.tensor_scalar(w[:], t3[:], 1.0, 1.0e30, OP.add, OP.min)
                r = work.tile([128, 2 * BS], f32, tag="r")
                nc.vector.reciprocal_approx_fast(r[:], w[:])
                g = work.tile([128, 2 * BS], mmdt, tag="g")
                nc.vector.scalar_tensor_tensor(
                    g[:], r[:], -0.5, dxb_t[:, c * 32:(c + 1) * 32],
                    OP.add, OP.mult)
                g_prev = g

        # ---- finish: y_T = y0 + sum_t Sel @ g_t ----
        nc.tensor.matmul(psum_y[:], ct["Sel"][:], g_prev[:, 0:BS],
                         start=False, stop=False, skip_group_check=True)
        nc.tensor.matmul(psum_y[:], ct["Sel"][:], g_prev[:, BS:2 * BS],
                         start=False, stop=True, skip_group_check=True)
        y_sb = work.tile([32, BS], f32, tag="y_sb")
        nc.scalar.activation(y_sb[:], psum_y[:], AF.Identity, bias=ct["b2c"][:])
        # readout
        psl = ptmp.tile([NCLS, BS], f32, tag="ptmp")
        nc.tensor.matmul(psl[:], ct["RT"][:], y_sb[:], start=True, stop=True)
        out_sb = work.tile([NCLS, BS], f32, tag="out_sb")
        nc.scalar.activation(out_sb[:], psl[:], AF.Identity, bias=ct["rbc"][:])
        nc.sync.dma_start(out_dram[:], out_sb[:])

    nc.compile()
    _NC_CACHE[key] = nc
    return nc


# --------------------------------------------------------------------------
# Public entry point
# --------------------------------------------------------------------------

def _prepare_inputs(ts, coeff_d, coeff_c, coeff_b, coeff_a,
                    W0, b0, W1, b1, W2, b2, F0, f0, F1, f1, F2, f2, R, rb,
                    num_steps, steps_per_chunk):
    ts = np.asarray(ts, dtype=_F32)
    coeff_a = np.asarray(coeff_a, dtype=_F32)
    dx = _spline_dx(ts, np.asarray(coeff_d, _F32), np.asarray(coeff_c, _F32),
                    np.asarray(coeff_b, _F32), num_steps)          # (S,B,D), dt folded
    W = _host_weights(*[np.asarray(a, _F32) for a in
                        (W0, b0, W1, b1, W2, b2, F0, f0, F1, f1, F2, f2, R, rb)])
    in_maps = []
    for core in range(NCORES):
        bs = slice(core * BS, (core + 1) * BS)
        m = dict(W)
        m["x0"] = np.ascontiguousarray(coeff_a[bs, 0, :].T)        # (8,16)
        m["dxb"] = _dxb_layout(dx[:, bs, :], steps_per_chunk)      # (CH,128,C*32)
        in_maps.append(m)
    return in_maps


def kernel(ts, coeff_d, coeff_c, coeff_b, coeff_a,
           W0, b0, W1, b1, W2, b2, F0, f0, F1, f1, F2, f2, R, rb):
    from concourse.bass_utils import run_bass_kernel_spmd

    num_steps = NUM_STEPS
    steps_per_chunk = 250
    nc = _build_nc(num_steps, steps_per_chunk)
    in_maps = _prepare_inputs(ts, coeff_d, coeff_c, coeff_b, coeff_a,
                              W0, b0, W1, b1, W2, b2, F0, f0, F1, f1, F2, f2,
                              R, rb, num_steps, steps_per_chunk)
    res = run_bass_kernel_spmd(nc, in_maps, list(range(NCORES)))
    logits = np.concatenate(
        [res.results[i]["logits"].T for i in range(NCORES)], axis=0)
    return np.ascontiguousarray(logits.astype(np.float32))
